# revision 15
# baseline (speedup 1.0000x reference)
"""Causal self-attention on 8 Trainium2 NeuronCores.

Sharding: core = (batch b in {0,1}) x (head-group g in {0..3}), 4 heads per
core. Each core computes qkv for its heads from x[b], runs causal attention,
and multiplies by its 256 rows of w_proj, producing a partial [T, C] output.
Host sums the 4 partials per batch.

Layout: everything is computed "transposed" so no on-chip transposes are
needed. The host feeds x[b].T; q^T/k^T come out of the qkv matmul with
head-dim on partitions (exactly the S^T = K Q^T operand layout); softmax is
done on S^T (keys on partitions, queries on free) with the denominator
obtained by appending a ones-column to V in the A@V matmul; the A@V output
Y^T is exactly the lhsT layout the final projection needs.

All matmuls run in float32r (full PE rate for free-dim >= 256, ~1.5e-4 rel
err). Producers write f32r via DMA-cast (gpsimd), DVE copy/mul, or ACT exp.
Causal masking is additive (-1e30) on the S^T PSUM tile before exp.
"""

import numpy as np

import concourse.bass as bass
import concourse.bacc as bacc
import concourse.tile as tile
from concourse import mybir
from concourse.bass_utils import run_bass_kernel_spmd

F32 = mybir.dt.float32
F32R = mybir.dt.float32r
BF16 = mybir.dt.bfloat16
EXP = mybir.ActivationFunctionType.Exp

B, T, C, H, HD = 2, 2048, 1024, 16, 64
NCORES = 8
HPC = 4      # heads per core
NPAIR = 2    # head pairs per core
NCT = C // 128   # 8 c-tiles
NTT = T // 128   # 16 t-tiles
NQC = T // 512   # 4 query chunks
SCALE = 1.0 / np.sqrt(HD)
NEG = -1.0e30


def build_kernel():
    nc = bacc.Bacc("TRN2", target_bir_lowering=False, debug=False, num_devices=NCORES)

    xT = nc.dram_tensor("xT", [C, T], F32, kind="ExternalInput")
    wqk = nc.dram_tensor("wqk", [C, 512], F32, kind="ExternalInput")
    wv = nc.dram_tensor("wv", [C, 256], F32, kind="ExternalInput")
    wp = nc.dram_tensor("wp", [256, C], F32, kind="ExternalInput")
    maskc = nc.dram_tensor("maskc", [128, 768], F32, kind="ExternalInput")
    sel = nc.dram_tensor("sel", [2, 128], F32, kind="ExternalInput")
    vones = nc.dram_tensor("vones", [128, 64], F32, kind="ExternalInput")
    out = nc.dram_tensor("out", [T, C], F32, kind="ExternalOutput")

    with tile.TileContext(nc) as tc:
        _body(tc, xT, wqk, wv, wp, maskc, sel, vones, out)

    nc.compile()
    return nc


def _body(tc, xT, wqk, wv, wp, maskc, sel, vones, out):
    nc = tc.nc
    from contextlib import ExitStack

    with ExitStack() as ctx:
        sb = lambda name: ctx.enter_context(tc.tile_pool(name=name, bufs=1))
        qkT_sb = sb("qkT").tile([128, 4 * T], BF16)       # bands q0,k0,q1,k1
        v65_sb = sb("v65").tile([128, NTT * 260], BF16)   # per k-tile: 4x(64 v + 1 ones)
        yt_sb = sb("yt").tile([128, NPAIR * T], BF16)     # pair p: rows 0-63 head 2p, 64-127 head 2p+1
        wp_sb = sb("wp").tile([128, 2 * C], BF16)
        maskc_sb = sb("maskc").tile([128, 768], F32)
        sel_sb = sb("sel").tile([2, 128], F32)

        es_pool = ctx.enter_context(tc.tile_pool(name="es", bufs=3))
        sums_pool = ctx.enter_context(tc.tile_pool(name="sums", bufs=2))
        rc_pool = ctx.enter_context(tc.tile_pool(name="rc", bufs=2))
        avst_pool = ctx.enter_context(tc.tile_pool(name="avst", bufs=2))
        ytr_pool = ctx.enter_context(tc.tile_pool(name="ytr", bufs=2))
        ost_pool = ctx.enter_context(tc.tile_pool(name="ost", bufs=2))

        # PSUM: psS tag = 2 slots x [128,1024] (4 banks), av 2 banks, misc 2
        ps = ctx.enter_context(tc.tile_pool(name="ps", bufs=2, space="PSUM"))
        av_pool = ctx.enter_context(tc.tile_pool(name="av", bufs=2, space="PSUM"))
        psS_pool = ctx.enter_context(tc.tile_pool(name="psS", bufs=2, space="PSUM"))

        nc.sync.dma_start(maskc_sb[:], maskc[:])
        nc.sync.dma_start(sel_sb[:], sel[:])
        for p in range(2):
            nc.gpsimd.dma_start(wp_sb[:, p * C:(p + 1) * C], wp[p * 128:(p + 1) * 128, :])
        v65_4d = v65_sb[:].rearrange("p (t h d) -> p t h d", t=NTT, h=HPC, d=65)
        nc.gpsimd.dma_start(
            v65_4d[:, :, :, 64:65],
            vones.ap().rearrange("p (t h o) -> p t h o", t=NTT, h=HPC, o=1))

        with tc.tile_pool(name="xw", bufs=1) as xw_pool:
            xT_sb = xw_pool.tile([128, NCT * T], BF16, name="xT_sb")
            wqk_sb = xw_pool.tile([128, NCT * 512], BF16, name="wqk_sb")
            wv_sb = xw_pool.tile([128, NCT * 256], BF16, name="wv_sb")
            for c in range(NCT):
                nc.gpsimd.dma_start(xT_sb[:, c * T:(c + 1) * T], xT[c * 128:(c + 1) * 128, :])
                nc.gpsimd.dma_start(wqk_sb[:, c * 512:(c + 1) * 512], wqk[c * 128:(c + 1) * 128, :])
                nc.gpsimd.dma_start(wv_sb[:, c * 256:(c + 1) * 256], wv[c * 128:(c + 1) * 128, :])

            # ---- stage A: qkT bands [128, T] = w_band^T @ xT ----
            for b in range(4):
                for tg in range(2):
                    acc = psS_pool.tile([128, 1024], F32, tag="psS", name=f"accA_{b}_{tg}")
                    for c in range(NCT):
                        lhs = wqk_sb[:, c * 512 + b * 128: c * 512 + (b + 1) * 128]
                        for half in range(2):
                            t4 = 2 * tg + half
                            nc.tensor.matmul(
                                acc[:, half * 512:(half + 1) * 512], lhs,
                                xT_sb[:, c * T + t4 * 512: c * T + (t4 + 1) * 512],
                                start=(c == 0), stop=(c == NCT - 1))
                    nc.vector.tensor_copy(qkT_sb[:, b * T + tg * 1024: b * T + (tg + 1) * 1024], acc[:])

            # ---- stage B: v natural [t, j] ----
            for t in range(NTT):
                psv = ps.tile([128, 512], F32, tag="ps", name=f"psv_{t}")
                for c in range(NCT):
                    lhs = xT_sb[:, c * T + t * 128: c * T + (t + 1) * 128]
                    nc.tensor.matmul(psv[:, 0:256], lhs, wv_sb[:, c * 256:(c + 1) * 256],
                                     start=(c == 0), stop=(c == NCT - 1))
                dst = v65_sb[:, t * 260:(t + 1) * 260].rearrange("p (h d) -> p h d", h=HPC, d=65)
                src = psv[:, 0:256].rearrange("p (h d) -> p h d", h=HPC, d=64)
                nc.vector.tensor_copy(dst[:, :, 0:64], src)

        # ---- stage C: attention; stage D: projection, lagged one qc and
        # drip-fed into the kt loop to keep the PE's duty cycle high (HAM
        # halves the PE clock if window-average activity drops below ~50%) --
        def emit_proj_group(t, n):
            pso = ps.tile([128, 512], F32, tag="ps", name=f"pso_{t}_{n}")
            for p in range(NPAIR):
                lhsT = yt_sb[:, p * T + t * 128: p * T + (t + 1) * 128]
                rhs = wp_sb[:, p * C + n * 512: p * C + (n + 1) * 512]
                nc.tensor.matmul(pso[:], lhsT, rhs, start=(p == 0), stop=(p == NPAIR - 1))
            ost = ost_pool.tile([128, 512], F32, tag="ost", name=f"ost_{t}_{n}")
            nc.vector.tensor_copy(ost[:], pso[:])
            nc.sync.dma_start(out[t * 128:(t + 1) * 128, n * 512:(n + 1) * 512], ost[:])

        pending_proj = []

        for qc in range(NQC):
            nkt = 4 * qc + 4
            ytrs, sumss = [], []
            for p in range(NPAIR):
                qb, kb = 2 * p, 2 * p + 1
                av = [av_pool.tile([128, 512], F32, tag="av", name=f"av_{p}_{qc}_{i}") for i in range(2)]

                def emit_S(kt, p=p, qb=qb, kb=kb, qc=qc):
                    psb = psS_pool.tile([128, 1024], F32, tag="psS", name=f"psS_{p}_{qc}_{kt}")
                    with tc.tile_critical():  # keep the row-pair adjacent on PE
                        for h in range(2):
                            base = 64 * h
                            lhsT = qkT_sb[base:base + 64, kb * T + kt * 128: kb * T + (kt + 1) * 128]
                            rhs = qkT_sb[base:base + 64, qb * T + qc * 512: qb * T + (qc + 1) * 512]
                            nc.tensor.matmul(psb[:, h * 512:(h + 1) * 512], lhsT, rhs,
                                             start=True, stop=True, tile_position=(base, 0))
                    return psb

                cur = emit_S(0)
                for kt in range(nkt):
                    nxt = emit_S(kt + 1) if kt + 1 < nkt else None
                    d = kt - 4 * qc
                    lo = max(d, 0) * 128  # first valid query column of this k-tile
                    psb2 = cur[:].rearrange("p (h q) -> p h q", h=2, q=512)
                    if d >= 0:
                        nc.vector.tensor_add(psb2[:, :, lo:lo + 128], psb2[:, :, lo:lo + 128],
                                             maskc_sb[:, 512:768].rearrange("p (h q) -> p h q", h=2, q=128))
                    es = es_pool.tile([128, 1024], BF16, tag="es", name=f"es_{p}_{qc}_{kt}")
                    es2 = es[:].rearrange("p (h q) -> p h q", h=2, q=512)
                    nc.scalar.activation(es2[:, :, lo:], psb2[:, :, lo:], EXP, scale=SCALE)
                    for h in range(2):
                        hh = 2 * p + h
                        lhsT_v = v65_sb[:, kt * 260 + hh * 65: kt * 260 + (hh + 1) * 65]
                        nc.tensor.matmul(av[h][0:65, lo:], lhsT_v, es[:, h * 512 + lo:(h + 1) * 512],
                                         start=(kt == 0), stop=(kt == nkt - 1))
                    if pending_proj and kt % 2 == 1:
                        emit_proj_group(*pending_proj.pop(0))
                    cur = nxt
                # evict Y^T + sums (PSUM can't feed DMA: stage via SBUF; the
                # partition shift for head b / sums rides the SBUF->SBUF DMA)
                ytr = ytr_pool.tile([128, 512], F32, tag="ytr", name=f"ytr_{p}_{qc}")
                sums = sums_pool.tile([2, 512], F32, tag="sums", name=f"sums_{p}_{qc}")
                for h in range(2):
                    st = avst_pool.tile([65, 512], F32, tag="avst", name=f"avst_{p}_{qc}_{h}")
                    nc.vector.tensor_copy(st[:], av[h][0:65, :])
                    nc.sync.dma_start(ytr[64 * h:64 * (h + 1), :], st[0:64, :])
                    nc.sync.dma_start(sums[h:h + 1, :], st[64:65, :])
                ytrs.append(ytr)
                sumss.append(sums)
            psRs = []
            for p in range(NPAIR):
                rc = rc_pool.tile([2, 512], F32, tag="rc", name=f"rc_{p}_{qc}")
                nc.vector.reciprocal(rc[:], sumss[p][:])
                psR = ps.tile([128, 512], F32, tag="ps", name=f"psR_{p}_{qc}")
                nc.tensor.matmul(psR[:], sel_sb[:], rc[:], start=True, stop=True)
                psRs.append(psR)
            for p in range(NPAIR):
                nc.vector.tensor_mul(yt_sb[:, p * T + qc * 512: p * T + (qc + 1) * 512],
                                     ytrs[p][:], psRs[p][:])
            pending_proj += [(t, n) for t in range(4 * qc, 4 * qc + 4) for n in range(2)]
        for t, n in pending_proj:
            emit_proj_group(t, n)



_NC_CACHE = None


def _get_nc():
    global _NC_CACHE
    if _NC_CACHE is None:
        _NC_CACHE = build_kernel()
    return _NC_CACHE


def _make_in_maps(x, w_attn, w_proj):
    x = np.asarray(x, dtype=np.float32)
    w_attn = np.asarray(w_attn, dtype=np.float32)
    w_proj = np.asarray(w_proj, dtype=np.float32)
    # maskc: cols 0:512 all NEG; cols 512:640 = strictly-lower-triangular NEG
    # (row j = key, col i = query; masked iff j > i)
    maskc = np.zeros((128, 768), dtype=np.float32)
    maskc[:, 0:512] = NEG
    tri = np.tril(np.full((128, 128), NEG, dtype=np.float32), -1)
    maskc[:, 512:640] = tri
    maskc[:, 640:768] = tri
    # mask is added before the exp's scale is applied, so pre-divide
    maskc /= SCALE
    sel = np.zeros((2, 128), dtype=np.float32)
    sel[0, 0:64] = 1.0
    sel[1, 64:128] = 1.0
    vones = np.ones((128, 64), dtype=np.float32)
    in_maps = []
    for core in range(NCORES):
        b, g = core // 4, core % 4
        hs = g * HPC
        q_cols = w_attn[:, hs * HD:(hs + HPC) * HD]
        k_cols = w_attn[:, C + hs * HD: C + (hs + HPC) * HD]
        v_cols = w_attn[:, 2 * C + hs * HD: 2 * C + (hs + HPC) * HD]
        wqk = np.concatenate(
            [q_cols[:, 0:128], k_cols[:, 0:128], q_cols[:, 128:256], k_cols[:, 128:256]], axis=1)
        in_maps.append({
            "xT": np.ascontiguousarray(x[b].T),
            "wqk": np.ascontiguousarray(wqk),
            "wv": np.ascontiguousarray(v_cols),
            "wp": np.ascontiguousarray(w_proj[hs * HD:(hs + HPC) * HD, :]),
            "maskc": maskc,
            "sel": sel,
            "vones": vones,
        })
    return in_maps


def run_cores(x, w_attn, w_proj, trace=False):
    nc = _get_nc()
    in_maps = _make_in_maps(x, w_attn, w_proj)
    res = run_bass_kernel_spmd(nc, in_maps, core_ids=list(range(NCORES)), trace=trace)
    out = np.zeros((B, T, C), dtype=np.float32)
    for core in range(NCORES):
        out[core // 4] += res.results[core]["out"]
    return out, res


def kernel(x, w_attn, w_proj):
    out, _ = run_cores(x, w_attn, w_proj, trace=False)
    return out


# revision 16
# speedup vs baseline: 1.5258x; 1.5258x over previous
"""Causal self-attention on 8 Trainium2 NeuronCores.

Sharding: core = (batch b in {0,1}) x (head-group g in {0..3}), 4 heads per
core. Each core computes qkv for its heads from x[b], runs causal attention,
and multiplies by its 256 rows of w_proj, producing a partial [T, C] output.
Host sums the 4 partials per batch.

Layout: everything is computed "transposed" so no on-chip transposes are
needed. The host feeds x[b].T; q^T/k^T come out of the qkv matmul with
head-dim on partitions (exactly the S^T = K Q^T operand layout); softmax is
done on S^T (keys on partitions, queries on free) with the denominator
obtained by appending a ones-column to V in the A@V matmul; the A@V output
Y^T is exactly the lhsT layout the final projection needs.

All matmuls run in float32r (full PE rate for free-dim >= 256, ~1.5e-4 rel
err). Producers write f32r via DMA-cast (gpsimd), DVE copy/mul, or ACT exp.
Causal masking is additive (-1e30) on the S^T PSUM tile before exp.
"""

import numpy as np

import concourse.bass as bass
import concourse.bacc as bacc
import concourse.tile as tile
from concourse import mybir
from concourse.bass_utils import run_bass_kernel_spmd

F32 = mybir.dt.float32
F32R = mybir.dt.float32r
BF16 = mybir.dt.bfloat16
EXP = mybir.ActivationFunctionType.Exp

B, T, C, H, HD = 2, 2048, 1024, 16, 64
NCORES = 8
HPC = 4      # heads per core
NPAIR = 2    # head pairs per core
NCT = C // 128   # 8 c-tiles
NTT = T // 128   # 16 t-tiles
NQC = T // 512   # 4 query chunks
SCALE = 1.0 / np.sqrt(HD)
NEG = -1.0e30


def build_kernel():
    nc = bacc.Bacc("TRN2", target_bir_lowering=False, debug=False, num_devices=NCORES)

    xT = nc.dram_tensor("xT", [C, T], F32, kind="ExternalInput")
    wqk = nc.dram_tensor("wqk", [C, 512], F32, kind="ExternalInput")
    wv = nc.dram_tensor("wv", [C, 256], F32, kind="ExternalInput")
    wp = nc.dram_tensor("wp", [256, C], F32, kind="ExternalInput")
    maskc = nc.dram_tensor("maskc", [128, 768], F32, kind="ExternalInput")
    sel = nc.dram_tensor("sel", [2, 128], F32, kind="ExternalInput")
    vones = nc.dram_tensor("vones", [128, 64], F32, kind="ExternalInput")
    out = nc.dram_tensor("out", [T, C], F32, kind="ExternalOutput")

    with tile.TileContext(nc) as tc:
        _body(tc, xT, wqk, wv, wp, maskc, sel, vones, out)

    nc.compile()
    return nc


def _body(tc, xT, wqk, wv, wp, maskc, sel, vones, out):
    nc = tc.nc
    from contextlib import ExitStack

    with ExitStack() as ctx:
        sb = lambda name: ctx.enter_context(tc.tile_pool(name=name, bufs=1))
        qkT_sb = sb("qkT").tile([128, 4 * T], BF16)       # bands q0,k0,q1,k1
        v65_sb = sb("v65").tile([128, NTT * 260], BF16)   # per k-tile: 4x(64 v + 1 ones)
        yt_sb = sb("yt").tile([128, NPAIR * T], BF16)     # pair p: rows 0-63 head 2p, 64-127 head 2p+1
        wp_sb = sb("wp").tile([128, 2 * C], BF16)
        maskc_sb = sb("maskc").tile([128, 768], F32)
        sel_sb = sb("sel").tile([2, 128], F32)

        es_pool = ctx.enter_context(tc.tile_pool(name="es", bufs=3))
        sums_pool = ctx.enter_context(tc.tile_pool(name="sums", bufs=2))
        rc_pool = ctx.enter_context(tc.tile_pool(name="rc", bufs=2))
        avst_pool = ctx.enter_context(tc.tile_pool(name="avst", bufs=2))
        ytr_pool = ctx.enter_context(tc.tile_pool(name="ytr", bufs=2))
        ost_pool = ctx.enter_context(tc.tile_pool(name="ost", bufs=2))

        # PSUM: psS tag = 2 slots x [128,1024] (4 banks), av 2 banks, misc 2
        ps = ctx.enter_context(tc.tile_pool(name="ps", bufs=2, space="PSUM"))
        av_pool = ctx.enter_context(tc.tile_pool(name="av", bufs=2, space="PSUM"))
        psS_pool = ctx.enter_context(tc.tile_pool(name="psS", bufs=2, space="PSUM"))

        nc.sync.dma_start(maskc_sb[:], maskc[:])
        nc.sync.dma_start(sel_sb[:], sel[:])
        for p in range(2):
            nc.gpsimd.dma_start(wp_sb[:, p * C:(p + 1) * C], wp[p * 128:(p + 1) * 128, :])
        v65_4d = v65_sb[:].rearrange("p (t h d) -> p t h d", t=NTT, h=HPC, d=65)
        nc.gpsimd.dma_start(
            v65_4d[:, :, :, 64:65],
            vones.ap().rearrange("p (t h o) -> p t h o", t=NTT, h=HPC, o=1))

        with tc.tile_pool(name="xw", bufs=1) as xw_pool:
            xT_sb = xw_pool.tile([128, NCT * T], BF16, name="xT_sb")
            wqk_sb = xw_pool.tile([128, NCT * 512], BF16, name="wqk_sb")
            wv_sb = xw_pool.tile([128, NCT * 256], BF16, name="wv_sb")
            for c in range(NCT):
                nc.gpsimd.dma_start(xT_sb[:, c * T:(c + 1) * T], xT[c * 128:(c + 1) * 128, :])
                nc.gpsimd.dma_start(wqk_sb[:, c * 512:(c + 1) * 512], wqk[c * 128:(c + 1) * 128, :])
                nc.gpsimd.dma_start(wv_sb[:, c * 256:(c + 1) * 256], wv[c * 128:(c + 1) * 128, :])

            # ---- stage A: qkT bands [128, T] = w_band^T @ xT ----
            for b in range(4):
                for tg in range(2):
                    acc = psS_pool.tile([128, 1024], F32, tag="psS", name=f"accA_{b}_{tg}")
                    for c in range(NCT):
                        lhs = wqk_sb[:, c * 512 + b * 128: c * 512 + (b + 1) * 128]
                        for half in range(2):
                            t4 = 2 * tg + half
                            nc.tensor.matmul(
                                acc[:, half * 512:(half + 1) * 512], lhs,
                                xT_sb[:, c * T + t4 * 512: c * T + (t4 + 1) * 512],
                                start=(c == 0), stop=(c == NCT - 1))
                    nc.vector.tensor_copy(qkT_sb[:, b * T + tg * 1024: b * T + (tg + 1) * 1024], acc[:])

            # ---- stage B: v natural [t, j] ----
            for t in range(NTT):
                psv = ps.tile([128, 512], F32, tag="ps", name=f"psv_{t}")
                for c in range(NCT):
                    lhs = xT_sb[:, c * T + t * 128: c * T + (t + 1) * 128]
                    nc.tensor.matmul(psv[:, 0:256], lhs, wv_sb[:, c * 256:(c + 1) * 256],
                                     start=(c == 0), stop=(c == NCT - 1))
                dst = v65_sb[:, t * 260:(t + 1) * 260].rearrange("p (h d) -> p h d", h=HPC, d=65)
                src = psv[:, 0:256].rearrange("p (h d) -> p h d", h=HPC, d=64)
                nc.vector.tensor_copy(dst[:, :, 0:64], src)

        # ---- stage C: attention; stage D: projection. All cross-chunk
        # serial work (normalization chain, projection) is drip-fed into the
        # NEXT chunk's kt loop so the in-order PE stream never stalls >3.4us
        # (a fully-idle HAM window would halve the PE clock for the rest of
        # the attention phase).
        def emit_proj_group(t, n):
            pso = ps.tile([128, 512], F32, tag="ps", name=f"pso_{t}_{n}")
            for p in range(NPAIR):
                lhsT = yt_sb[:, p * T + t * 128: p * T + (t + 1) * 128]
                rhs = wp_sb[:, p * C + n * 512: p * C + (n + 1) * 512]
                nc.tensor.matmul(pso[:], lhsT, rhs, start=(p == 0), stop=(p == NPAIR - 1))
            ost = ost_pool.tile([128, 512], F32, tag="ost", name=f"ost_{t}_{n}")
            nc.vector.tensor_copy(ost[:], pso[:])
            nc.sync.dma_start(out[t * 128:(t + 1) * 128, n * 512:(n + 1) * 512], ost[:])

        pending = []  # deferred closures: normalization + projection work

        for qc in range(NQC):
            nkt = 4 * qc + 4
            ytrs, sumss, rcs = [], [], []
            for p in range(NPAIR):
                qb, kb = 2 * p, 2 * p + 1
                av = [av_pool.tile([128, 512], F32, tag="av", name=f"av_{p}_{qc}_{i}") for i in range(2)]

                def emit_S(kt, p=p, qb=qb, kb=kb, qc=qc):
                    psb = psS_pool.tile([128, 1024], F32, tag="psS", name=f"psS_{p}_{qc}_{kt}")
                    for h in range(2):
                        base = 64 * h
                        lhsT = qkT_sb[base:base + 64, kb * T + kt * 128: kb * T + (kt + 1) * 128]
                        rhs = qkT_sb[base:base + 64, qb * T + qc * 512: qb * T + (qc + 1) * 512]
                        nc.tensor.matmul(psb[:, h * 512:(h + 1) * 512], lhsT, rhs,
                                         start=True, stop=True, tile_position=(base, 0))
                    return psb

                pipe = [emit_S(0)]
                if nkt > 1:
                    pipe.append(emit_S(1))
                for kt in range(nkt):
                    cur = pipe.pop(0)
                    if kt + 2 < nkt:
                        pipe.append(emit_S(kt + 2))
                    d = kt - 4 * qc
                    lo = max(d, 0) * 128  # first valid query column of this k-tile
                    psb2 = cur[:].rearrange("p (h q) -> p h q", h=2, q=512)
                    if d >= 0:
                        nc.vector.tensor_add(psb2[:, :, lo:lo + 128], psb2[:, :, lo:lo + 128],
                                             maskc_sb[:, 512:768].rearrange("p (h q) -> p h q", h=2, q=128))
                    es = es_pool.tile([128, 1024], BF16, tag="es", name=f"es_{p}_{qc}_{kt}")
                    es2 = es[:].rearrange("p (h q) -> p h q", h=2, q=512)
                    nc.scalar.activation(es2[:, :, lo:], psb2[:, :, lo:], EXP, scale=SCALE)
                    for h in range(2):
                        hh = 2 * p + h
                        lhsT_v = v65_sb[:, kt * 260 + hh * 65: kt * 260 + (hh + 1) * 65]
                        nc.tensor.matmul(av[h][0:65, lo:], lhsT_v, es[:, h * 512 + lo:(h + 1) * 512],
                                         start=(kt == 0), stop=(kt == nkt - 1))
                    if pending and kt >= 2:
                        pending.pop(0)()
                # evict Y^T + sums (PSUM can't feed DMA: stage via SBUF; the
                # partition shift for head b / sums rides the SBUF->SBUF DMA)
                ytr = ytr_pool.tile([128, 512], F32, tag="ytr", name=f"ytr_{p}_{qc}")
                sums = sums_pool.tile([2, 512], F32, tag="sums", name=f"sums_{p}_{qc}")
                for h in range(2):
                    st = avst_pool.tile([65, 512], F32, tag="avst", name=f"avst_{p}_{qc}_{h}")
                    nc.vector.tensor_copy(st[:], av[h][0:65, :])
                    nc.sync.dma_start(ytr[64 * h:64 * (h + 1), :], st[0:64, :])
                    nc.sync.dma_start(sums[h:h + 1, :], st[64:65, :])
                ytrs.append(ytr)
                sumss.append(sums)
            for p in range(NPAIR):
                rc = rc_pool.tile([2, 512], F32, tag="rc", name=f"rc_{p}_{qc}")
                nc.vector.reciprocal(rc[:], sumss[p][:])
                rcs.append(rc)

            def norm_pair(p, qc=qc, ytrs=ytrs, rcs=rcs):
                psR = ps.tile([128, 512], F32, tag="ps", name=f"psR_{p}_{qc}")
                nc.tensor.matmul(psR[:], sel_sb[:], rcs[p][:], start=True, stop=True)
                nc.vector.tensor_mul(yt_sb[:, p * T + qc * 512: p * T + (qc + 1) * 512],
                                     ytrs[p][:], psR[:])

            pending += [lambda p=p: norm_pair(p) for p in range(NPAIR)]
            pending += [lambda t=t, n=n: emit_proj_group(t, n)
                        for t in range(4 * qc, 4 * qc + 4) for n in range(2)]
        for fn in pending:
            fn()



_NC_CACHE = None


def _get_nc():
    global _NC_CACHE
    if _NC_CACHE is None:
        _NC_CACHE = build_kernel()
    return _NC_CACHE


def _make_in_maps(x, w_attn, w_proj):
    x = np.asarray(x, dtype=np.float32)
    w_attn = np.asarray(w_attn, dtype=np.float32)
    w_proj = np.asarray(w_proj, dtype=np.float32)
    # maskc: cols 0:512 all NEG; cols 512:640 = strictly-lower-triangular NEG
    # (row j = key, col i = query; masked iff j > i)
    maskc = np.zeros((128, 768), dtype=np.float32)
    maskc[:, 0:512] = NEG
    tri = np.tril(np.full((128, 128), NEG, dtype=np.float32), -1)
    maskc[:, 512:640] = tri
    maskc[:, 640:768] = tri
    # mask is added before the exp's scale is applied, so pre-divide
    maskc /= SCALE
    sel = np.zeros((2, 128), dtype=np.float32)
    sel[0, 0:64] = 1.0
    sel[1, 64:128] = 1.0
    vones = np.ones((128, 64), dtype=np.float32)
    in_maps = []
    for core in range(NCORES):
        b, g = core // 4, core % 4
        hs = g * HPC
        q_cols = w_attn[:, hs * HD:(hs + HPC) * HD]
        k_cols = w_attn[:, C + hs * HD: C + (hs + HPC) * HD]
        v_cols = w_attn[:, 2 * C + hs * HD: 2 * C + (hs + HPC) * HD]
        wqk = np.concatenate(
            [q_cols[:, 0:128], k_cols[:, 0:128], q_cols[:, 128:256], k_cols[:, 128:256]], axis=1)
        in_maps.append({
            "xT": np.ascontiguousarray(x[b].T),
            "wqk": np.ascontiguousarray(wqk),
            "wv": np.ascontiguousarray(v_cols),
            "wp": np.ascontiguousarray(w_proj[hs * HD:(hs + HPC) * HD, :]),
            "maskc": maskc,
            "sel": sel,
            "vones": vones,
        })
    return in_maps


def run_cores(x, w_attn, w_proj, trace=False):
    nc = _get_nc()
    in_maps = _make_in_maps(x, w_attn, w_proj)
    res = run_bass_kernel_spmd(nc, in_maps, core_ids=list(range(NCORES)), trace=trace)
    out = np.zeros((B, T, C), dtype=np.float32)
    for core in range(NCORES):
        out[core // 4] += res.results[core]["out"]
    return out, res


def kernel(x, w_attn, w_proj):
    out, _ = run_cores(x, w_attn, w_proj, trace=False)
    return out


# revision 18
# speedup vs baseline: 1.5421x; 1.0107x over previous
"""Causal self-attention on 8 Trainium2 NeuronCores.

Sharding: core = (batch b in {0,1}) x (head-group g in {0..3}), 4 heads per
core. Each core computes qkv for its heads from x[b], runs causal attention,
and multiplies by its 256 rows of w_proj, producing a partial [T, C] output.
Host sums the 4 partials per batch.

Layout: everything is computed "transposed" so no on-chip transposes are
needed. The host feeds x[b].T; q^T/k^T come out of the qkv matmul with
head-dim on partitions (exactly the S^T = K Q^T operand layout); softmax is
done on S^T (keys on partitions, queries on free) with the denominator
obtained by appending a ones-column to V in the A@V matmul; the A@V output
Y^T is exactly the lhsT layout the final projection needs.

All matmuls run in float32r (full PE rate for free-dim >= 256, ~1.5e-4 rel
err). Producers write f32r via DMA-cast (gpsimd), DVE copy/mul, or ACT exp.
Causal masking is additive (-1e30) on the S^T PSUM tile before exp.
"""

import numpy as np

import concourse.bass as bass
import concourse.bacc as bacc
import concourse.tile as tile
from concourse import mybir
from concourse.bass_utils import run_bass_kernel_spmd

F32 = mybir.dt.float32
F32R = mybir.dt.float32r
BF16 = mybir.dt.bfloat16
EXP = mybir.ActivationFunctionType.Exp

B, T, C, H, HD = 2, 2048, 1024, 16, 64
NCORES = 8
HPC = 4      # heads per core
NPAIR = 2    # head pairs per core
NCT = C // 128   # 8 c-tiles
NTT = T // 128   # 16 t-tiles
NQC = T // 512   # 4 query chunks
SCALE = 1.0 / np.sqrt(HD)
NEG = -1.0e30


def build_kernel():
    nc = bacc.Bacc("TRN2", target_bir_lowering=False, debug=False, num_devices=NCORES)

    xT = nc.dram_tensor("xT", [C, T], F32, kind="ExternalInput")
    wqk = nc.dram_tensor("wqk", [C, 512], F32, kind="ExternalInput")
    wv = nc.dram_tensor("wv", [C, 256], F32, kind="ExternalInput")
    wp = nc.dram_tensor("wp", [256, C], F32, kind="ExternalInput")
    maskc = nc.dram_tensor("maskc", [128, 768], F32, kind="ExternalInput")
    sel = nc.dram_tensor("sel", [2, 128], F32, kind="ExternalInput")
    vones = nc.dram_tensor("vones", [128, 64], F32, kind="ExternalInput")
    out = nc.dram_tensor("out", [T, C], F32, kind="ExternalOutput")

    with tile.TileContext(nc) as tc:
        _body(tc, xT, wqk, wv, wp, maskc, sel, vones, out)

    nc.compile()
    return nc


def _body(tc, xT, wqk, wv, wp, maskc, sel, vones, out):
    nc = tc.nc
    from contextlib import ExitStack

    with ExitStack() as ctx:
        sb = lambda name: ctx.enter_context(tc.tile_pool(name=name, bufs=1))
        qkT_sb = sb("qkT").tile([128, 4 * T], BF16)       # bands q0,k0,q1,k1
        v65_sb = sb("v65").tile([128, NTT * 260], BF16)   # per k-tile: 4x(64 v + 1 ones)
        yt_sb = sb("yt").tile([128, NPAIR * T], BF16)     # pair p: rows 0-63 head 2p, 64-127 head 2p+1
        wp_sb = sb("wp").tile([128, 2 * C], BF16)
        maskc_sb = sb("maskc").tile([128, 768], F32)
        sel_sb = sb("sel").tile([2, 128], F32)

        es_pool = ctx.enter_context(tc.tile_pool(name="es", bufs=3))
        sums_pool = ctx.enter_context(tc.tile_pool(name="sums", bufs=2))
        rc_pool = ctx.enter_context(tc.tile_pool(name="rc", bufs=2))
        avst_pool = ctx.enter_context(tc.tile_pool(name="avst", bufs=2))
        ytr_pool = ctx.enter_context(tc.tile_pool(name="ytr", bufs=2))
        ost_pool = ctx.enter_context(tc.tile_pool(name="ost", bufs=2))

        # PSUM: psS tag = 2 slots x [128,1024] (4 banks), av 2 banks, misc 2
        ps = ctx.enter_context(tc.tile_pool(name="ps", bufs=2, space="PSUM"))
        av_pool = ctx.enter_context(tc.tile_pool(name="av", bufs=2, space="PSUM"))
        psS_pool = ctx.enter_context(tc.tile_pool(name="psS", bufs=2, space="PSUM"))

        nc.sync.dma_start(maskc_sb[:], maskc[:])
        nc.sync.dma_start(sel_sb[:], sel[:])
        for p in range(2):
            nc.gpsimd.dma_start(wp_sb[:, p * C:(p + 1) * C], wp[p * 128:(p + 1) * 128, :])
        v65_4d = v65_sb[:].rearrange("p (t h d) -> p t h d", t=NTT, h=HPC, d=65)
        nc.gpsimd.dma_start(
            v65_4d[:, :, :, 64:65],
            vones.ap().rearrange("p (t h o) -> p t h o", t=NTT, h=HPC, o=1))

        with tc.tile_pool(name="xw", bufs=1) as xw_pool:
            xT_sb = xw_pool.tile([128, NCT * T], BF16, name="xT_sb")
            wqk_sb = xw_pool.tile([128, NCT * 512], BF16, name="wqk_sb")
            wv_sb = xw_pool.tile([128, NCT * 256], BF16, name="wv_sb")
            for c in range(NCT):
                nc.gpsimd.dma_start(wqk_sb[:, c * 512:(c + 1) * 512], wqk[c * 128:(c + 1) * 128, :])
                nc.gpsimd.dma_start(wv_sb[:, c * 256:(c + 1) * 256], wv[c * 128:(c + 1) * 128, :])
            for c in range(NCT):
                for hf in range(2):
                    nc.gpsimd.dma_start(
                        xT_sb[:, c * T + hf * 1024: c * T + (hf + 1) * 1024],
                        xT[c * 128:(c + 1) * 128, hf * 1024:(hf + 1) * 1024])

            # ---- stage A: qkT bands [128, T] = w_band^T @ xT ----
            # 4 concurrent t-chunk accumulators per band (the psS slots plus
            # the av slots, idle until stage C) so each lhsT load feeds 4 MMs
            # and the PE never waits on an eviction.
            for b in range(4):
                acc01 = psS_pool.tile([128, 1024], F32, tag="psS", name=f"accA_{b}_01")
                acc2 = av_pool.tile([128, 512], F32, tag="av", name=f"accA_{b}_2")
                acc3 = av_pool.tile([128, 512], F32, tag="av", name=f"accA_{b}_3")
                dsts = [acc01[:, 0:512], acc01[:, 512:1024], acc2[:], acc3[:]]
                for c in range(NCT):
                    lhs = wqk_sb[:, c * 512 + b * 128: c * 512 + (b + 1) * 128]
                    for t4 in range(4):
                        nc.tensor.matmul(
                            dsts[t4], lhs,
                            xT_sb[:, c * T + t4 * 512: c * T + (t4 + 1) * 512],
                            start=(c == 0), stop=(c == NCT - 1))
                nc.vector.tensor_copy(qkT_sb[:, b * T: b * T + 1024], acc01[:])
                nc.vector.tensor_copy(qkT_sb[:, b * T + 1024: b * T + 1536], acc2[:])
                nc.vector.tensor_copy(qkT_sb[:, b * T + 1536: b * T + 2048], acc3[:])

            # ---- stage B: v natural [t, j] ----
            for t in range(NTT):
                psv = ps.tile([128, 512], F32, tag="ps", name=f"psv_{t}")
                for c in range(NCT):
                    lhs = xT_sb[:, c * T + t * 128: c * T + (t + 1) * 128]
                    nc.tensor.matmul(psv[:, 0:256], lhs, wv_sb[:, c * 256:(c + 1) * 256],
                                     start=(c == 0), stop=(c == NCT - 1))
                dst = v65_sb[:, t * 260:(t + 1) * 260].rearrange("p (h d) -> p h d", h=HPC, d=65)
                src = psv[:, 0:256].rearrange("p (h d) -> p h d", h=HPC, d=64)
                nc.vector.tensor_copy(dst[:, :, 0:64], src)

        # ---- stage C: attention; stage D: projection. All cross-chunk
        # serial work (normalization chain, projection) is drip-fed into the
        # NEXT chunk's kt loop so the in-order PE stream never stalls >3.4us
        # (a fully-idle HAM window would halve the PE clock for the rest of
        # the attention phase).
        def emit_proj_group(t, n):
            pso = ps.tile([128, 512], F32, tag="ps", name=f"pso_{t}_{n}")
            for p in range(NPAIR):
                lhsT = yt_sb[:, p * T + t * 128: p * T + (t + 1) * 128]
                rhs = wp_sb[:, p * C + n * 512: p * C + (n + 1) * 512]
                nc.tensor.matmul(pso[:], lhsT, rhs, start=(p == 0), stop=(p == NPAIR - 1))
            ost = ost_pool.tile([128, 512], F32, tag="ost", name=f"ost_{t}_{n}")
            nc.vector.tensor_copy(ost[:], pso[:])
            nc.sync.dma_start(out[t * 128:(t + 1) * 128, n * 512:(n + 1) * 512], ost[:])

        pending = []  # deferred closures: normalization + projection work

        for qc in range(NQC):
            nkt = 4 * qc + 4
            ytrs, sumss, rcs = [], [], []
            for p in range(NPAIR):
                qb, kb = 2 * p, 2 * p + 1
                av = [av_pool.tile([128, 512], F32, tag="av", name=f"av_{p}_{qc}_{i}") for i in range(2)]

                def emit_S(kt, p=p, qb=qb, kb=kb, qc=qc):
                    psb = psS_pool.tile([128, 1024], F32, tag="psS", name=f"psS_{p}_{qc}_{kt}")
                    for h in range(2):
                        base = 64 * h
                        lhsT = qkT_sb[base:base + 64, kb * T + kt * 128: kb * T + (kt + 1) * 128]
                        rhs = qkT_sb[base:base + 64, qb * T + qc * 512: qb * T + (qc + 1) * 512]
                        nc.tensor.matmul(psb[:, h * 512:(h + 1) * 512], lhsT, rhs,
                                         start=True, stop=True, tile_position=(base, 0))
                    return psb

                pipe = [emit_S(0)]
                if nkt > 1:
                    pipe.append(emit_S(1))
                for kt in range(nkt):
                    cur = pipe.pop(0)
                    if kt + 2 < nkt:
                        pipe.append(emit_S(kt + 2))
                    d = kt - 4 * qc
                    lo = max(d, 0) * 128  # first valid query column of this k-tile
                    psb2 = cur[:].rearrange("p (h q) -> p h q", h=2, q=512)
                    if d >= 0:
                        nc.vector.tensor_add(psb2[:, :, lo:lo + 128], psb2[:, :, lo:lo + 128],
                                             maskc_sb[:, 512:768].rearrange("p (h q) -> p h q", h=2, q=128))
                    es = es_pool.tile([128, 1024], BF16, tag="es", name=f"es_{p}_{qc}_{kt}")
                    es2 = es[:].rearrange("p (h q) -> p h q", h=2, q=512)
                    nc.scalar.activation(es2[:, :, lo:], psb2[:, :, lo:], EXP, scale=SCALE)
                    for h in range(2):
                        hh = 2 * p + h
                        lhsT_v = v65_sb[:, kt * 260 + hh * 65: kt * 260 + (hh + 1) * 65]
                        nc.tensor.matmul(av[h][0:65, lo:], lhsT_v, es[:, h * 512 + lo:(h + 1) * 512],
                                         start=(kt == 0), stop=(kt == nkt - 1))
                    if pending and kt >= 2:
                        pending.pop(0)()
                # evict Y^T + sums (PSUM can't feed DMA: stage via SBUF; the
                # partition shift for head b / sums rides the SBUF->SBUF DMA)
                ytr = ytr_pool.tile([128, 512], F32, tag="ytr", name=f"ytr_{p}_{qc}")
                sums = sums_pool.tile([2, 512], F32, tag="sums", name=f"sums_{p}_{qc}")
                for h in range(2):
                    st = avst_pool.tile([65, 512], F32, tag="avst", name=f"avst_{p}_{qc}_{h}")
                    nc.vector.tensor_copy(st[:], av[h][0:65, :])
                    nc.sync.dma_start(ytr[64 * h:64 * (h + 1), :], st[0:64, :])
                    nc.sync.dma_start(sums[h:h + 1, :], st[64:65, :])
                ytrs.append(ytr)
                sumss.append(sums)
            for p in range(NPAIR):
                rc = rc_pool.tile([2, 512], F32, tag="rc", name=f"rc_{p}_{qc}")
                nc.vector.reciprocal(rc[:], sumss[p][:])
                rcs.append(rc)

            def norm_pair(p, qc=qc, ytrs=ytrs, rcs=rcs):
                psR = ps.tile([128, 512], F32, tag="ps", name=f"psR_{p}_{qc}")
                nc.tensor.matmul(psR[:], sel_sb[:], rcs[p][:], start=True, stop=True)
                nc.vector.tensor_mul(yt_sb[:, p * T + qc * 512: p * T + (qc + 1) * 512],
                                     ytrs[p][:], psR[:])

            pending += [lambda p=p: norm_pair(p) for p in range(NPAIR)]
            pending += [lambda t=t, n=n: emit_proj_group(t, n)
                        for t in range(4 * qc, 4 * qc + 4) for n in range(2)]
        for fn in pending:
            fn()



_NC_CACHE = None


def _get_nc():
    global _NC_CACHE
    if _NC_CACHE is None:
        _NC_CACHE = build_kernel()
    return _NC_CACHE


def _make_in_maps(x, w_attn, w_proj):
    x = np.asarray(x, dtype=np.float32)
    w_attn = np.asarray(w_attn, dtype=np.float32)
    w_proj = np.asarray(w_proj, dtype=np.float32)
    # maskc: cols 0:512 all NEG; cols 512:640 = strictly-lower-triangular NEG
    # (row j = key, col i = query; masked iff j > i)
    maskc = np.zeros((128, 768), dtype=np.float32)
    maskc[:, 0:512] = NEG
    tri = np.tril(np.full((128, 128), NEG, dtype=np.float32), -1)
    maskc[:, 512:640] = tri
    maskc[:, 640:768] = tri
    # mask is added before the exp's scale is applied, so pre-divide
    maskc /= SCALE
    sel = np.zeros((2, 128), dtype=np.float32)
    sel[0, 0:64] = 1.0
    sel[1, 64:128] = 1.0
    vones = np.ones((128, 64), dtype=np.float32)
    in_maps = []
    for core in range(NCORES):
        b, g = core // 4, core % 4
        hs = g * HPC
        q_cols = w_attn[:, hs * HD:(hs + HPC) * HD]
        k_cols = w_attn[:, C + hs * HD: C + (hs + HPC) * HD]
        v_cols = w_attn[:, 2 * C + hs * HD: 2 * C + (hs + HPC) * HD]
        wqk = np.concatenate(
            [q_cols[:, 0:128], k_cols[:, 0:128], q_cols[:, 128:256], k_cols[:, 128:256]], axis=1)
        in_maps.append({
            "xT": np.ascontiguousarray(x[b].T),
            "wqk": np.ascontiguousarray(wqk),
            "wv": np.ascontiguousarray(v_cols),
            "wp": np.ascontiguousarray(w_proj[hs * HD:(hs + HPC) * HD, :]),
            "maskc": maskc,
            "sel": sel,
            "vones": vones,
        })
    return in_maps


def run_cores(x, w_attn, w_proj, trace=False):
    nc = _get_nc()
    in_maps = _make_in_maps(x, w_attn, w_proj)
    res = run_bass_kernel_spmd(nc, in_maps, core_ids=list(range(NCORES)), trace=trace)
    out = np.zeros((B, T, C), dtype=np.float32)
    for core in range(NCORES):
        out[core // 4] += res.results[core]["out"]
    return out, res


def kernel(x, w_attn, w_proj):
    out, _ = run_cores(x, w_attn, w_proj, trace=False)
    return out
